# revision 4
# baseline (speedup 1.0000x reference)
"""MoE AlltoAllTokenDispatcher kernel for TRN2 (8 NeuronCores).

The reference dispatcher's gather (tokens[argsort(idx)//k]) followed by
scatter-add at the same argsort permutation is an exact identity on slot
order: unpermuted[s] == tokens[s // k] for every slot s, independent of the
routing indices. The whole module therefore reduces to

    out[i] = tokens[i] * (probs[i, 0] + probs[i, 1])

which is a pure memory-bound row-scaling. We shard the token dimension
across the 8 cores (data-parallel, per the sharding hint's token-dim
sharding; no all-to-all is needed since expert compute is identity).

Pipeline per core (raw Bass, manual semaphores — avoids the Tile drain
barrier):
  - sync engine  (qSP HWDGE ring):  probs gather, then token tile loads
  - vector engine (DVE):            scale reduce once, per-tile scalar mul
  - scalar engine (qAct HWDGE ring): token tile stores
Loads and stores live on different HWDGE rings so a store waiting on
compute never blocks a later load's dispatch.
"""

import numpy as np

import concourse.bass as bass
import concourse.tile as tile
from concourse import bacc, mybir
from concourse.bass_utils import run_bass_kernel_spmd

N_TOKENS = 16384
HIDDEN = 4096
TOP_K = 2
N_CORES = 8
TOK_PER_CORE = N_TOKENS // N_CORES  # 2048
P = 128
N_TILES = TOK_PER_CORE // P  # 16
N_BUFS = 6

_nc_cache = None


def _work_items():
    """(tile_idx, col_start, ncols) per pipeline step; first and last row
    tiles are split in half along hidden to shorten ramp and tail."""
    items = []
    h2 = HIDDEN // 2
    for i in range(N_TILES):
        if i in (0, N_TILES - 1):
            items.append((i, 0, h2))
            items.append((i, h2, h2))
        else:
            items.append((i, 0, HIDDEN))
    return items


def _build_nc_raw():
    nc = bass.Bass("TRN2", target_bir_lowering=False, debug=False)
    tokens = nc.dram_tensor(
        "tokens", [TOK_PER_CORE, HIDDEN], mybir.dt.float32, kind="ExternalInput"
    )
    probs = nc.dram_tensor(
        "probs", [TOK_PER_CORE, TOP_K], mybir.dt.float32, kind="ExternalInput"
    )
    out = nc.dram_tensor(
        "out", [TOK_PER_CORE, HIDDEN], mybir.dt.float32, kind="ExternalOutput"
    )

    items = _work_items()
    n_items = len(items)

    with (
        nc.sbuf_tensor("buf", [P, N_BUFS * HIDDEN], mybir.dt.float32) as buf,
        nc.sbuf_tensor("pt", [P, N_TILES * TOP_K], mybir.dt.float32) as pt,
        nc.sbuf_tensor("st", [P, N_TILES], mybir.dt.float32) as st,
        nc.semaphore("probs_sem") as probs_sem,
        nc.semaphore("load_sem") as load_sem,
        nc.semaphore("comp_sem") as comp_sem,
        nc.semaphore("store_sem") as store_sem,
        nc.Block() as block,
    ):

        def slot(j, ncols):
            base = (j % N_BUFS) * HIDDEN
            return buf[:, base : base + ncols]

        @block.sync
        def _(sync: bass.BassEngine):
            sync.dma_start(
                out=pt[:].rearrange("p (n k) -> p n k", k=TOP_K),
                in_=probs.rearrange("(n p) k -> p n k", p=P),
            ).then_inc(probs_sem, 16)
            for j, (i, c0, ncols) in enumerate(items):
                if j >= N_BUFS:
                    sync.wait_ge(store_sem, 16 * (j - N_BUFS + 1))
                sync.dma_start(
                    out=slot(j, ncols),
                    in_=tokens[i * P : (i + 1) * P, c0 : c0 + ncols],
                ).then_inc(load_sem, 16)

        @block.vector
        def _(vector: bass.BassEngine):
            vector.wait_ge(probs_sem, 16)
            vector.reduce_sum(
                st[:],
                pt[:].rearrange("p (n k) -> p n k", k=TOP_K),
                axis=mybir.AxisListType.X,
            )
            for j, (i, c0, ncols) in enumerate(items):
                vector.wait_ge(load_sem, 16 * (j + 1))
                vector.tensor_scalar_mul(
                    slot(j, ncols), slot(j, ncols), st[:, i : i + 1]
                ).then_inc(comp_sem, 1)

        @block.scalar
        def _(scalar: bass.BassEngine):
            for j, (i, c0, ncols) in enumerate(items):
                scalar.wait_ge(comp_sem, j + 1)
                scalar.dma_start(
                    out=out[i * P : (i + 1) * P, c0 : c0 + ncols],
                    in_=slot(j, ncols),
                ).then_inc(store_sem, 16)
            scalar.wait_ge(store_sem, 16 * n_items)

    return nc


def _build_nc_tile():
    """Tile-framework fallback (v2): ~170 us vs raw pipeline."""
    nc = bacc.Bacc(
        "TRN2", target_bir_lowering=False, debug=False, num_devices=N_CORES
    )
    tokens = nc.dram_tensor(
        "tokens", [TOK_PER_CORE, HIDDEN], mybir.dt.float32, kind="ExternalInput"
    ).ap()
    probs = nc.dram_tensor(
        "probs", [TOK_PER_CORE, TOP_K], mybir.dt.float32, kind="ExternalInput"
    ).ap()
    out = nc.dram_tensor(
        "out", [TOK_PER_CORE, HIDDEN], mybir.dt.float32, kind="ExternalOutput"
    ).ap()

    with tile.TileContext(nc) as tc:
        with (
            tc.tile_pool(name="tok", bufs=N_BUFS) as tok_pool,
            tc.tile_pool(name="pr", bufs=1) as pr_pool,
            tc.tile_pool(name="sc", bufs=1) as sc_pool,
        ):
            pt = pr_pool.tile([P, N_TILES * TOP_K], mybir.dt.float32)
            st = sc_pool.tile([P, N_TILES], mybir.dt.float32)
            nc.sync.dma_start(
                out=pt[:].rearrange("p (n k) -> p n k", k=TOP_K),
                in_=probs.rearrange("(n p) k -> p n k", p=P),
            )
            nc.vector.reduce_sum(
                st[:],
                pt[:].rearrange("p (n k) -> p n k", k=TOP_K),
                axis=mybir.AxisListType.X,
            )
            for i, c0, ncols in _work_items():
                tt = tok_pool.tile([P, ncols], mybir.dt.float32, tag="tok")
                nc.sync.dma_start(
                    out=tt[:, :ncols],
                    in_=tokens[i * P : (i + 1) * P, c0 : c0 + ncols],
                )
                nc.vector.tensor_scalar_mul(
                    tt[:, :ncols], tt[:, :ncols], st[:, i : i + 1]
                )
                nc.scalar.dma_start(
                    out=out[i * P : (i + 1) * P, c0 : c0 + ncols],
                    in_=tt[:, :ncols],
                )
    nc.compile()
    return nc


def _build():
    import os

    if os.environ.get("KERNEL_VARIANT", "raw") == "tile":
        return _build_nc_tile()
    return _build_nc_raw()


def kernel(tokens, probs, indices=None, **_unused):
    global _nc_cache
    tokens = np.ascontiguousarray(np.asarray(tokens, dtype=np.float32))
    probs = np.ascontiguousarray(np.asarray(probs, dtype=np.float32))
    assert tokens.shape == (N_TOKENS, HIDDEN)
    assert probs.shape == (N_TOKENS, TOP_K)

    if _nc_cache is None:
        _nc_cache = _build()
    nc = _nc_cache

    in_maps = [
        {
            "tokens": tokens[c * TOK_PER_CORE : (c + 1) * TOK_PER_CORE],
            "probs": probs[c * TOK_PER_CORE : (c + 1) * TOK_PER_CORE],
        }
        for c in range(N_CORES)
    ]
    res = run_bass_kernel_spmd(nc, in_maps, core_ids=list(range(N_CORES)))
    return np.concatenate([res.results[c]["out"] for c in range(N_CORES)], axis=0)
